# revision 1
# baseline (speedup 1.0000x reference)
"""CTC greedy decode kernel for Trainium2 (Bass/Tile), 8-core data-parallel.

Problem: log_probs [32, 4096, 1025] f32, input_lengths [32] i64 ->
  preds    [32, 4096] int32  (per-frame argmax)
  keep     [32, 4096] bool   (non-blank & != prev & t < len)
  max_logp [32, 4096] f32    (value at argmax)

Sharding: batch dim across 8 cores (4 utterances each). Per core:
16384 frames x 1025 vocab. Frames ride the SBUF partition dim (128
frames/tile, 128 tiles); vocab rides the free dim. Per tile the DVE
computes reduce_max (value) then max_index (argmax, first occurrence =
jnp.argmax tie-break). The CTC collapse mask is a handful of small
elementwise ops on the [128, 128] per-core result grid.
"""

from contextlib import nullcontext

import numpy as np

import concourse.bacc as bacc
import concourse.mybir as mybir
from concourse.tile import TileContext
from concourse.bass_utils import run_bass_kernel_spmd

B, T, V = 32, 4096, 1025
BLANK = 1024
NCORES = 8
BLOC = B // NCORES        # utterances per core
F = BLOC * T              # frames per core
P = 128                   # partitions
NT = F // P               # tiles per core (128)
CPU = T // P              # columns per utterance (32)
G = 2                     # tiles per DMA batch / batched reduce
NB = NT // G

_CACHE = {}


def _build_program(repeat=1, mode="mi4b", g=G, bufs=8):
    if mode in ("mi4", "mi4b"):
        g = 4
    if mode == "mi8b":
        g, bufs = 8, 3
    nc = bacc.Bacc(None, target_bir_lowering=False)
    f32 = mybir.dt.float32
    lp = nc.dram_tensor("lp", [F, V], f32, kind="ExternalInput")
    tv = nc.dram_tensor("tv", [P, NT], f32, kind="ExternalInput")
    ln = nc.dram_tensor("ln", [P, NT], f32, kind="ExternalInput")
    off = nc.dram_tensor("off", [P, NT], f32, kind="ExternalInput")
    preds_o = nc.dram_tensor("preds", [P, NT], mybir.dt.int32, kind="ExternalOutput")
    keep_o = nc.dram_tensor("keep", [P, NT], mybir.dt.int32, kind="ExternalOutput")
    mlp_o = nc.dram_tensor("maxlp", [P, NT], f32, kind="ExternalOutput")

    # frame f = n*128 + p  ->  [p, n, v]
    lp_r = lp.rearrange("(n p) v -> p n v", p=P)

    with TileContext(nc) as tc:
        with (
            tc.tile_pool(name="loads", bufs=bufs) as loads,
            tc.tile_pool(name="persist", bufs=1) as pp,
        ):
            NGRP = NT // 4
            NBIG = NT // 8
            nstage = {"mi4": NGRP, "mi4b": NGRP, "mi8b": NGRP}.get(mode, NT)
            stage = pp.tile([P, 8 * nstage], mybir.dt.uint32)
            stage3 = stage.rearrange("p (r c) -> p r c", c=nstage)
            gmax = pp.tile([P, NT], f32)
            offt = pp.tile([P, NT], f32)
            tvt = pp.tile([P, NT], f32)
            lnt = pp.tile([P, NT], f32)
            preds_f = pp.tile([P, NT], f32)
            prev_f = pp.tile([P, NT], f32)
            preds_i = pp.tile([P, NT], mybir.dt.int32)
            k1 = pp.tile([P, NT], f32)
            k2 = pp.tile([P, NT], f32)
            keep_i = pp.tile([P, NT], mybir.dt.int32)

            nc.sync.dma_start(out=tvt[:], in_=tv[:])
            nc.sync.dma_start(out=lnt[:], in_=ln[:])
            nc.sync.dma_start(out=offt[:], in_=off[:])
            # valid mask is loop-independent: compute it up front
            nc.vector.tensor_tensor(
                out=k2[:], in0=tvt[:], in1=lnt[:], op=mybir.AluOpType.is_lt
            )
            if mode == "nored":
                nc.vector.memset(gmax[:], 0.0)
            if mode == "nomi":
                nc.vector.memset(stage[:], 0)

            nb = NT // g
            loop_cm = tc.For_i(0, repeat, 1) if repeat > 1 else nullcontext()
            with loop_cm:
                if mode == "mi8b":
                    # 8-tile big: one batched reduce + two 4-tile max_index
                    # scans. Needles for both scans = the big's 8 tile-maxes;
                    # the out-of-scan half of the needle window is ignored, so
                    # slot = tile % 8 uniformly. Exactness requires no tile's
                    # max to occur bit-exactly in an earlier tile of its
                    # 4-tile scan (verified zero such collisions).
                    for m in range(NBIG):
                        i0 = m * 8
                        big = loads.tile([P, 8, V], f32, tag="big")
                        nc.sync.dma_start(
                            out=big[:, 0:4, :], in_=lp_r[:, i0 : i0 + 4, :]
                        )
                        nc.sync.dma_start(
                            out=big[:, 4:8, :], in_=lp_r[:, i0 + 4 : i0 + 8, :]
                        )
                        nc.vector.tensor_reduce(
                            out=gmax[:, i0 : i0 + 8],
                            in_=big[:],
                            axis=mybir.AxisListType.X,
                            op=mybir.AluOpType.max,
                        )
                        needles = gmax[:, i0 : i0 + 8]
                        flat = big.rearrange("p g v -> p (g v)")
                        # scan s of big m -> stage column 2m+s; useful slots
                        # are 0:4 for s=0, 4:8 for s=1 (slot = tile%8)
                        nc.vector.max_index(
                            out=stage3[:, :, 2 * m],
                            in_max=needles,
                            in_values=flat[:, 0 : 4 * V],
                        )
                        nc.vector.max_index(
                            out=stage3[:, :, 2 * m + 1],
                            in_max=needles,
                            in_values=flat[:, 4 * V : 8 * V],
                        )
                if mode == "mi4":
                    # One max_index scan per 4-tile group (8200-cycle scan,
                    # 8 needles = this group's 4 maxes + next group's 4;
                    # slots for the next group are ignored). Needle window
                    # [i0:i0+8] for groups 0..30, [NT-8:NT] for the last.
                    prev_big = None
                    for grp in range(NGRP):
                        i0 = grp * 4
                        big = loads.tile([P, 4, V], f32, tag="big")
                        nc.sync.dma_start(out=big[:], in_=lp_r[:, i0 : i0 + 4, :])
                        nc.vector.tensor_reduce(
                            out=gmax[:, i0 : i0 + 4],
                            in_=big[:],
                            axis=mybir.AxisListType.X,
                            op=mybir.AluOpType.max,
                        )
                        if grp >= 1:
                            w0 = (grp - 1) * 4
                            nc.vector.max_index(
                                out=stage3[:, :, grp - 1],
                                in_max=gmax[:, w0 : w0 + 8],
                                in_values=prev_big.rearrange("p g v -> p (g v)"),
                            )
                        prev_big = big
                    nc.vector.max_index(
                        out=stage3[:, :, NGRP - 1],
                        in_max=gmax[:, NT - 8 : NT],
                        in_values=prev_big.rearrange("p g v -> p (g v)"),
                    )
                if mode == "mi4b":
                    # Backward needle window [i0-4:i0+4]: MI(grp) depends only
                    # on reduces already emitted (in-scan needles in slots
                    # 4:8); group 0 uses [0:8] with in-scan slots 0:4.
                    # Group 0 is sub-tiled (per-tile DMA + reduce) so the DVE
                    # starts after the first 525KB instead of the full 2.1MB.
                    for grp in range(NGRP):
                        i0 = grp * 4
                        big = loads.tile([P, 4, V], f32, tag="big")
                        if grp == 0:
                            for k in range(4):
                                nc.sync.dma_start(
                                    out=big[:, k, :], in_=lp_r[:, i0 + k, :]
                                )
                                nc.vector.tensor_reduce(
                                    out=gmax[:, i0 + k : i0 + k + 1],
                                    in_=big[:, k, :],
                                    axis=mybir.AxisListType.X,
                                    op=mybir.AluOpType.max,
                                )
                        else:
                            nc.sync.dma_start(
                                out=big[:], in_=lp_r[:, i0 : i0 + 4, :]
                            )
                            nc.vector.tensor_reduce(
                                out=gmax[:, i0 : i0 + 4],
                                in_=big[:],
                                axis=mybir.AxisListType.X,
                                op=mybir.AluOpType.max,
                            )
                        w0 = 0 if grp == 0 else i0 - 4
                        nc.vector.max_index(
                            out=stage3[:, :, grp],
                            in_max=gmax[:, w0 : w0 + 8],
                            in_values=big.rearrange("p g v -> p (g v)"),
                        )
                for blk in range(0 if mode in ("mi4", "mi4b", "mi8b") else nb):
                    i0 = blk * g
                    big = loads.tile([P, g, V], f32, tag="big")
                    nc.sync.dma_start(out=big[:], in_=lp_r[:, i0 : i0 + g, :])
                    if mode == "evenred":
                        # even-width reduce (possible 2x perf mode) + fixup
                        nc.vector.tensor_reduce(
                            out=gmax[:, i0 : i0 + g],
                            in_=big[:, :, 0 : V - 1],
                            axis=mybir.AxisListType.X,
                            op=mybir.AluOpType.max,
                        )
                        nc.vector.tensor_tensor(
                            out=gmax[:, i0 : i0 + g],
                            in0=gmax[:, i0 : i0 + g],
                            in1=big[:, :, V - 1],
                            op=mybir.AluOpType.max,
                        )
                    elif mode == "poolred":
                        for k in range(g):
                            nc.vector.pool_max(
                                out=gmax[:, i0 + k : i0 + k + 1],
                                in_=big[:, k, :],
                            )
                    elif mode == "nored":
                        pass  # timing-only: skip max pass (wrong results)
                    else:
                        nc.vector.tensor_reduce(
                            out=gmax[:, i0 : i0 + g],
                            in_=big[:],
                            axis=mybir.AxisListType.X,
                            op=mybir.AluOpType.max,
                        )
                    if mode != "nomi":
                        for k in range(g):
                            i = i0 + k
                            nc.vector.max_index(
                                out=stage3[:, :, i],
                                in_max=gmax[:, i : i + 1].to_broadcast([P, 8]),
                                in_values=big[:, k, :],
                            )

            if mode in ("mi4", "mi4b", "mi8b"):
                # extract per-tile absolute indices from stage[p, slot, grp],
                # then subtract (tile%4)*V to localize within the tile.
                sel = stage.rearrange("p (r c) -> p c r", c=NGRP)  # [p, grp, slot]
                pf3 = preds_f.rearrange("p (g k) -> p g k", k=4)
                if mode == "mi4":
                    # slot k for groups 0..30, slot 4+k for the last group
                    nc.vector.tensor_copy(
                        out=pf3[:, 0 : NGRP - 1, :], in_=sel[:, 0 : NGRP - 1, 0:4]
                    )
                    nc.vector.tensor_copy(
                        out=pf3[:, NGRP - 1, :], in_=sel[:, NGRP - 1, 4:8]
                    )
                elif mode == "mi4b":
                    # slot k for group 0, slot 4+k for groups 1..31
                    nc.vector.tensor_copy(out=pf3[:, 0, :], in_=sel[:, 0, 0:4])
                    nc.vector.tensor_copy(
                        out=pf3[:, 1:NGRP, :], in_=sel[:, 1:NGRP, 4:8]
                    )
                else:
                    # slot = tile%8: even groups use slots 0:4, odd 4:8
                    sel4 = stage.rearrange("p (r c d) -> p c d r", r=8, d=2)
                    pf4 = preds_f.rearrange("p (g d k) -> p g d k", g=NBIG, d=2, k=4)
                    nc.vector.tensor_copy(
                        out=pf4[:, :, 0, :], in_=sel4[:, :, 0, 0:4]
                    )
                    nc.vector.tensor_copy(
                        out=pf4[:, :, 1, :], in_=sel4[:, :, 1, 4:8]
                    )
                nc.vector.tensor_tensor(
                    out=preds_f[:], in0=preds_f[:], in1=offt[:],
                    op=mybir.AluOpType.subtract,
                )
                nc.vector.tensor_copy(out=preds_i[:], in_=preds_f[:])
            else:
                # preds: rank-0 plane of stage, contiguous [P, NT] uint32
                top = stage[:, 0:NT]
                nc.vector.tensor_copy(out=preds_i[:], in_=top)
                nc.vector.tensor_copy(out=preds_f[:], in_=top)

            # prev (partition-shifted preds) via small SBUF->SBUF DMAs
            nc.sync.dma_start(out=prev_f[1:P, :], in_=preds_f[0 : P - 1, :])
            nc.sync.dma_start(out=prev_f[0:1, 1:NT], in_=preds_f[P - 1 : P, 0 : NT - 1])
            # sentinel -1 at utterance starts (cols 0, 32, 64, 96)
            sent = prev_f.rearrange("p (u c) -> p u c", c=CPU)[0:1, :, 0:1]
            nc.vector.memset(sent, -1.0)

            # blank-compare folds into k2 while the prev DMAs are in
            # flight; only ne(prev) + one mult remain on the serial tail
            nc.vector.tensor_scalar(
                out=k1[:], in0=preds_f[:], scalar1=float(BLANK), scalar2=None,
                op0=mybir.AluOpType.not_equal,
            )
            nc.vector.tensor_tensor(
                out=k2[:], in0=k1[:], in1=k2[:], op=mybir.AluOpType.mult
            )
            nc.vector.tensor_tensor(
                out=k1[:], in0=preds_f[:], in1=prev_f[:], op=mybir.AluOpType.not_equal
            )
            nc.vector.tensor_tensor(
                out=keep_i[:], in0=k1[:], in1=k2[:], op=mybir.AluOpType.mult
            )

            nc.sync.dma_start(out=preds_o[:], in_=preds_i[:])
            nc.sync.dma_start(out=keep_o[:], in_=keep_i[:])
            nc.sync.dma_start(out=mlp_o[:], in_=gmax[:])
    nc.compile()
    return nc


def _host_inputs(log_probs, input_lengths):
    log_probs = np.ascontiguousarray(np.asarray(log_probs, dtype=np.float32))
    lens = np.asarray(input_lengths).astype(np.int64)
    # tv[p, col] = within-utterance frame index of (p, col)
    cols = np.arange(NT)
    tvals = ((cols % CPU)[None, :] * P + np.arange(P)[:, None]).astype(np.float32)
    # off[p, col] = (col%4)*V: scan offset of tile col within its 4-tile group
    offs = np.broadcast_to(((cols % 4) * V).astype(np.float32)[None, :], (P, NT))
    offs = np.ascontiguousarray(offs, dtype=np.float32)
    in_maps = []
    for c in range(NCORES):
        lp_c = log_probs[c * BLOC : (c + 1) * BLOC].reshape(F, V)
        ln_c = lens[c * BLOC : (c + 1) * BLOC].astype(np.float32)
        ln_exp = np.broadcast_to(ln_c[cols // CPU][None, :], (P, NT))
        in_maps.append(
            {
                "lp": lp_c,
                "tv": tvals,
                "ln": np.ascontiguousarray(ln_exp, dtype=np.float32),
                "off": offs,
            }
        )
    return in_maps


def _grid_to_bt(arr):
    # arr [P, NT]: value for frame t=(col%32)*128+p of utterance col//32
    return arr.reshape(P, BLOC, CPU).transpose(1, 2, 0).reshape(BLOC, T)


def kernel(log_probs, input_lengths, **_kw):
    if "nc" not in _CACHE:
        _CACHE["nc"] = _build_program()
    nc = _CACHE["nc"]
    in_maps = _host_inputs(log_probs, input_lengths)
    res = run_bass_kernel_spmd(nc, in_maps, core_ids=list(range(NCORES)))
    preds = np.empty((B, T), dtype=np.int32)
    keep = np.empty((B, T), dtype=bool)
    max_logp = np.empty((B, T), dtype=np.float32)
    for c, r in enumerate(res.results):
        sl = slice(c * BLOC, (c + 1) * BLOC)
        preds[sl] = _grid_to_bt(r["preds"])
        keep[sl] = _grid_to_bt(r["keep"]).astype(bool)
        max_logp[sl] = _grid_to_bt(r["maxlp"])
    return preds, keep, max_logp



# revision 19
# speedup vs baseline: 1.1561x; 1.1561x over previous
"""CTC greedy decode kernel for Trainium2 (Bass/Tile), 8-core data-parallel.

Problem: log_probs [32, 4096, 1025] f32, input_lengths [32] i64 ->
  preds    [32, 4096] int32  (per-frame argmax)
  keep     [32, 4096] bool   (non-blank & != prev & t < len)
  max_logp [32, 4096] f32    (value at argmax)

Sharding: batch dim across 8 cores (4 utterances each). Per core:
16384 frames x 1025 vocab. Frames ride the SBUF partition dim (128
frames/tile, 128 tiles); vocab rides the free dim.

v3 ("split") design, from the v2 trace (217us; DMA packed 3.9..190.5us
but a 27us serial endgame):
 - Frame->grid mapping t = p*32 + c (c = col%32) so `prev` is a free-dim
   column shift (cheap DVE copy) instead of a full partition-shift DMA;
   only the 4 utterance-start cols {0,32,64,96} need a tiny
   partition-shift fixup DMA (col u*32 <- col u*32+31).
 - The per-(row,tile) max needles come from Pool (GPSIMD) elementwise-max
   binary trees for 24 groups (gpsimd.tensor_reduce can't reduce the free
   axis, so the tree is 11 tensor_tensor max levels, ~6.7us/group);
   DVE reduces 5 mid groups (pool backlog resets) + sub-tiled groups
   0/30/31. DVE also runs all max_index scans. Both engines sit under
   the ~190us DMA roofline instead of DVE serializing 2 passes (273us).
 - Group 31 is per-tile (DMA order tile 127 first) with broadcast-needle
   scans, so preds col 127 lands early and the col-96 fixup DMA overlaps
   the tail of the input stream.
 - The collapse-mask tail is split: cols 0:96 (A-chain) and 96:124 (B1)
   complete mid-stream; only cols {96} & 124:128 keep-ops trail the last
   scan (~2us tail).
 - tv/ln/off aux loads are squeezed mid-stream so the first big input
   DMA starts ~2us earlier; tail/output DMAs issue from the idle Act
   engine queue to avoid head-of-line blocking on SP.
"""

from contextlib import nullcontext

import numpy as np

import concourse.bacc as bacc
import concourse.mybir as mybir
from concourse.tile import TileContext
from concourse.bass_utils import run_bass_kernel_spmd

B, T, V = 32, 4096, 1025
BLANK = 1024
NCORES = 8
BLOC = B // NCORES        # utterances per core
F = BLOC * T              # frames per core
P = 128                   # partitions
NT = F // P               # tiles per core (128)
CPU = T // P              # columns per utterance (32)
NGRP = NT // 4            # 4-tile groups (32)

_CACHE = {}


def _build_program(repeat=1, mode="v3", bufs=9):
    nc = bacc.Bacc(None, target_bir_lowering=False)
    f32 = mybir.dt.float32
    mx = mybir.AluOpType.max
    X = mybir.AxisListType.X
    lp = nc.dram_tensor("lp", [F, V], f32, kind="ExternalInput")
    tv = nc.dram_tensor("tv", [P, NT], f32, kind="ExternalInput")
    ln = nc.dram_tensor("ln", [P, NT], f32, kind="ExternalInput")
    idm = nc.dram_tensor("idm", [P, P], f32, kind="ExternalInput")
    preds_o = nc.dram_tensor("preds", [P, NT], mybir.dt.int32, kind="ExternalOutput")
    keep_o = nc.dram_tensor("keep", [P, NT], mybir.dt.int32, kind="ExternalOutput")
    mlp_o = nc.dram_tensor("maxlp", [P, NT], f32, kind="ExternalOutput")

    # frame (u*4096 + p*32 + c) -> [p, u, c, v]; col = u*32+c, so prev is
    # a free-dim col shift
    lp_r4 = lp.rearrange("(u p c) v -> p u c v", u=BLOC, p=P, c=CPU)

    def lp_slice(i0, n):
        u, c = i0 // CPU, i0 % CPU
        return lp_r4[:, u, c : c + n, :]

    NST = NT              # one stage col per tile (per-tile scans)
    DVE_RED = (5, 13, 21, 27)

    with TileContext(nc) as tc:
        with (
            tc.tile_pool(name="loads", bufs=bufs) as loads,
            tc.tile_pool(name="temps", bufs=3) as temps,
            tc.tile_pool(name="persist", bufs=1) as pp,
            tc.psum_pool(name="pspool", bufs=2) as pspool,
        ):
            stage = pp.tile([P, 8 * NST], mybir.dt.uint32)
            stage3 = stage.rearrange("p (r c) -> p r c", c=NST)
            sel = stage.rearrange("p (r c) -> p c r", c=NST)  # [p, col, slot]
            gmax = pp.tile([P, NT], f32)
            tvt = pp.tile([P, NT], f32)
            lnt = pp.tile([P, NT], f32)
            preds_f = pp.tile([P, NT], f32)
            prev_f = pp.tile([P, NT], f32)
            preds_i = pp.tile([P, NT], mybir.dt.int32)
            k1 = pp.tile([P, NT], f32)
            k2 = pp.tile([P, NT], f32)
            keep_i = pp.tile([P, NT], mybir.dt.int32)
            preds4 = preds_f.rearrange("p (u c) -> p u c", c=CPU)
            prev4 = prev_f.rearrange("p (u c) -> p u c", c=CPU)
            pf1 = preds_f.rearrange("p (c one) -> p c one", one=1)
            sel1 = sel[:, :, 0:1]  # slot-0 plane: per-tile argmax indices

            ident = pp.tile([P, P], f32)

            def pe_needles(big, i0):
                # Per-tile free-axis max without touching the DVE's big
                # passes: PE transposes each [128f,128v] block into PSUM,
                # Act copies PSUM->SBUF (GPSIMD can't read PSUM), Pool
                # max-reduces the partition (vocab) axis -- its only legal
                # max -- and 32 tiny PE back-transposes return the chunk
                # partials to frame-major, where a tiny DVE reduce plus a
                # fold of the leftover col 1024 yields gmax[:, i0:i0+4].
                nrow = temps.tile([1, 4096], f32, tag="nrow")
                for k in range(8):
                    psA = pspool.tile([P, 4, P], f32, tag="psA")
                    for n in range(4):
                        nc.tensor.transpose(
                            out=psA[:, n, :],
                            in_=big[:, n, k * P : (k + 1) * P],
                            identity=ident[:],
                        )
                    sA = temps.tile([P, 4, P], f32, tag="sA")
                    nc.scalar.copy(out=sA[:], in_=psA[:])
                    nc.gpsimd.tensor_reduce(
                        out=nrow[0:1, k * 512 : (k + 1) * 512],
                        in_=sA.rearrange("p n f -> p (n f)"),
                        axis=mybir.AxisListType.C,
                        op=mx,
                    )
                psC = pspool.tile([P, 4, 8], f32, tag="psC")
                for n in range(4):
                    for k in range(8):
                        s = k * 512 + n * P
                        nc.tensor.transpose(
                            out=psC[:, n, k : k + 1],
                            in_=nrow[0:1, s : s + P],
                            identity=ident[0:1, 0:1],
                        )
                nc.vector.tensor_reduce(
                    out=gmax[:, i0 : i0 + 4], in_=psC[:], axis=X, op=mx
                )
                nc.vector.tensor_tensor(
                    out=gmax[:, i0 : i0 + 4], in0=gmax[:, i0 : i0 + 4],
                    in1=big[:, :, 1024], op=mx,
                )

            def dve_reduce(big, i0, k=None):
                if k is None:
                    nc.vector.tensor_reduce(
                        out=gmax[:, i0 : i0 + 4], in_=big[:], axis=X, op=mx
                    )
                else:
                    nc.vector.tensor_reduce(
                        out=gmax[:, i0 + k : i0 + k + 1],
                        in_=big[:, k, :],
                        axis=X,
                        op=mx,
                    )

            def tile_scans(big, i0, ks=range(4)):
                # Broadcast-needle per-tile scans: indices are tile-local
                # and immune to bit-exact cross-tile needle collisions (an
                # equal earlier value within the tile IS the argmax).
                for k in ks:
                    nc.vector.max_index(
                        out=stage3[:, :, i0 + k],
                        in_max=gmax[:, i0 + k : i0 + k + 1].to_broadcast(
                            [P, 8]
                        ),
                        in_values=big[:, k, :],
                    )

            loop_cm = tc.For_i(0, repeat, 1) if repeat > 1 else nullcontext()
            with loop_cm:
                for grp in range(NGRP):
                    i0 = grp * 4
                    big = loads.tile([P, 4, V], f32, tag="big")
                    if grp == NGRP - 1:
                        # per-tile endgame, tile 127 first so the col-96
                        # prev fixup DMA overlaps the remaining stream
                        for k in (3, 0, 1, 2):
                            nc.sync.dma_start(
                                out=big[:, k, :], in_=lp_slice(i0 + k, 1)[:, 0, :]
                            )
                            dve_reduce(big, i0, k)
                            tile_scans(big, i0, ks=(k,))
                            # per-tile extract (indices are tile-local)
                            nc.gpsimd.tensor_copy(
                                out=preds_f[:, i0 + k : i0 + k + 1],
                                in_=sel[:, i0 + k, 0:1],
                            )
                            if k == 3:
                                # prev col 96 <- preds col 127 (part shift)
                                nc.scalar.dma_start(
                                    out=prev4[1:P, 3:4, 0:1],
                                    in_=preds4[0 : P - 1, 3:4, 31:32],
                                )
                    elif grp in (0, 30):
                        for k in range(4):
                            nc.sync.dma_start(
                                out=big[:, k, :], in_=lp_slice(i0 + k, 1)[:, 0, :]
                            )
                            dve_reduce(big, i0, k)
                            tile_scans(big, i0, ks=(k,))
                    elif grp in DVE_RED:
                        nc.sync.dma_start(out=big[:], in_=lp_slice(i0, 4))
                        dve_reduce(big, i0)
                        tile_scans(big, i0)
                    else:
                        # two half-loads so the PE transposes can start
                        # ~2.9us before the group finishes loading
                        nc.sync.dma_start(
                            out=big[:, 0:2, :], in_=lp_slice(i0, 2)
                        )
                        nc.sync.dma_start(
                            out=big[:, 2:4, :], in_=lp_slice(i0 + 2, 2)
                        )
                        pe_needles(big, i0)
                        tile_scans(big, i0)

                    if grp == 0:
                        # identity for PE transposes, needed from group 1 on
                        nc.sync.dma_start(out=ident[:], in_=idm[:])
                    if grp == 1:
                        # aux loads squeezed mid-stream (needed only by the
                        # late keep-mask ops)
                        nc.sync.dma_start(out=tvt[:], in_=tv[:])
                        nc.sync.dma_start(out=lnt[:], in_=ln[:])
                        nc.vector.tensor_tensor(
                            out=k2[:], in0=tvt[:], in1=lnt[:],
                            op=mybir.AluOpType.is_lt,
                        )

                    if grp == 23:
                        _emit_chain_a(nc, sel1, pf1, preds_f, preds4, prev4,
                                      prev_f, preds_i, k1, k2, keep_i,
                                      preds_o, keep_o, mlp_o, gmax)
                    if grp == 29:
                        _emit_chain_b1(nc, sel1, pf1, preds_f, preds4, prev4,
                                       prev_f, preds_i, k1, k2, keep_i,
                                       preds_o, keep_o, mlp_o, gmax)

            _emit_fix(nc, sel1, pf1, preds_f, preds4, prev4, prev_f,
                      preds_i, k1, k2, keep_i, preds_o, keep_o, mlp_o, gmax)
    nc.compile()
    return nc


def _emit_chain_a(nc, sel1, pf1, preds_f, preds4, prev4, prev_f,
                  preds_i, k1, k2, keep_i, preds_o, keep_o, mlp_o, gmax):
    """Finalize cols 0:96 (utterances 0..2) mid-stream."""
    ne = mybir.AluOpType.not_equal
    mult = mybir.AluOpType.mult
    nc.vector.tensor_copy(out=pf1[:, 0:96, :], in_=sel1[:, 0:96, :])
    # prev: free-dim shift within each utterance
    nc.vector.tensor_copy(out=prev4[:, 0:3, 1:32], in_=preds4[:, 0:3, 0:31])
    # utterance-start cols {0,32,64} <- cols {31,63,95} of partition p-1
    nc.scalar.dma_start(out=prev4[1:P, 0:3, 0:1], in_=preds4[0 : P - 1, 0:3, 31:32])
    # t=0 sentinel for all 4 utterances (p=0 rows; col 96's p>=1 comes
    # from the late fixup DMA)
    nc.vector.memset(prev4[0:1, :, 0:1], -1.0)
    nc.vector.tensor_copy(out=preds_i[:, 0:96], in_=preds_f[:, 0:96])
    nc.vector.tensor_scalar(
        out=k1[:, 0:96], in0=preds_f[:, 0:96], scalar1=float(BLANK),
        scalar2=None, op0=ne,
    )
    nc.vector.tensor_tensor(
        out=k2[:, 0:96], in0=k1[:, 0:96], in1=k2[:, 0:96], op=mult
    )
    nc.vector.tensor_tensor(
        out=k1[:, 0:96], in0=preds_f[:, 0:96], in1=prev_f[:, 0:96], op=ne
    )
    nc.vector.tensor_tensor(
        out=keep_i[:, 0:96], in0=k1[:, 0:96], in1=k2[:, 0:96], op=mult
    )
    nc.scalar.dma_start(out=preds_o[:, 0:96], in_=preds_i[:, 0:96])
    nc.scalar.dma_start(out=keep_o[:, 0:96], in_=keep_i[:, 0:96])
    nc.scalar.dma_start(out=mlp_o[:, 0:96], in_=gmax[:, 0:96])


def _emit_chain_b1(nc, sel1, pf1, preds_f, preds4, prev4, prev_f,
                   preds_i, k1, k2, keep_i, preds_o, keep_o, mlp_o, gmax):
    """Cols 96:120 (utterance 3 head) on the Pool engine once the col-119
    scan is done; the DVE is saturated with scans at this point. Pool has
    no not_equal ALU op, so ne(a,b) is min((a-b)^2, 1) -- exact for these
    integer-valued floats."""
    sub = mybir.AluOpType.subtract
    mult = mybir.AluOpType.mult
    mn = mybir.AluOpType.min
    g = nc.gpsimd
    g.tensor_copy(out=pf1[:, 96:120, :], in_=sel1[:, 96:120, :])
    # prev cols 97:121 <- preds 96:120 (free-dim shift)
    g.tensor_copy(out=prev4[:, 3:4, 1:25], in_=preds4[:, 3:4, 0:24])
    g.tensor_copy(out=preds_i[:, 96:120], in_=preds_f[:, 96:120])
    g.tensor_scalar(
        out=k1[:, 96:120], in0=preds_f[:, 96:120], scalar1=float(BLANK),
        scalar2=None, op0=sub,
    )
    g.tensor_tensor(
        out=k1[:, 96:120], in0=k1[:, 96:120], in1=k1[:, 96:120], op=mult
    )
    g.tensor_scalar(
        out=k1[:, 96:120], in0=k1[:, 96:120], scalar1=1.0, scalar2=None,
        op0=mn,
    )
    g.tensor_tensor(
        out=k2[:, 96:120], in0=k1[:, 96:120], in1=k2[:, 96:120], op=mult
    )
    g.tensor_tensor(
        out=k1[:, 97:120], in0=preds_f[:, 97:120], in1=prev_f[:, 97:120],
        op=sub,
    )
    g.tensor_tensor(
        out=k1[:, 97:120], in0=k1[:, 97:120], in1=k1[:, 97:120], op=mult
    )
    g.tensor_scalar(
        out=k1[:, 97:120], in0=k1[:, 97:120], scalar1=1.0, scalar2=None,
        op0=mn,
    )
    # Pool integer TensorTensor needs matching dtypes: mult into f32,
    # then copy-convert to the int32 output tile
    g.tensor_tensor(
        out=k1[:, 97:120], in0=k1[:, 97:120], in1=k2[:, 97:120], op=mult
    )
    g.tensor_copy(out=keep_i[:, 97:120], in_=k1[:, 97:120])
    nc.scalar.dma_start(out=preds_o[:, 96:120], in_=preds_i[:, 96:120])
    nc.scalar.dma_start(out=keep_o[:, 97:120], in_=keep_i[:, 97:120])
    nc.scalar.dma_start(out=mlp_o[:, 96:120], in_=gmax[:, 96:120])


def _emit_fix(nc, sel1, pf1, preds_f, preds4, prev4, prev_f,
              preds_i, k1, k2, keep_i, preds_o, keep_o, mlp_o, gmax):
    """Final cols 120:128 (group 30 + per-tile group 31) plus the col-96
    keep (its prev comes from the early fixup DMA). On DVE -- all scans
    are done by the time these run, so the DVE is idle."""
    ne = mybir.AluOpType.not_equal
    mult = mybir.AluOpType.mult
    v = nc.vector
    # col 96: prev = fixup DMA (p>=1) + sentinel (p=0); k2 from B1
    v.tensor_tensor(
        out=k1[:, 96:97], in0=preds_f[:, 96:97], in1=prev_f[:, 96:97], op=ne
    )
    v.tensor_tensor(
        out=keep_i[:, 96:97], in0=k1[:, 96:97], in1=k2[:, 96:97], op=mult
    )
    nc.scalar.dma_start(out=keep_o[:, 96:97], in_=keep_i[:, 96:97])
    # cols 120:124 extract (group 30); cols 124:128 are per-tile extracts
    v.tensor_copy(out=pf1[:, 120:124, :], in_=sel1[:, 120:124, :])
    # prev cols 121:128 <- preds 120:127
    v.tensor_copy(out=prev4[:, 3:4, 25:32], in_=preds4[:, 3:4, 24:31])
    v.tensor_copy(out=preds_i[:, 120:128], in_=preds_f[:, 120:128])
    v.tensor_scalar(
        out=k1[:, 120:128], in0=preds_f[:, 120:128], scalar1=float(BLANK),
        scalar2=None, op0=ne,
    )
    v.tensor_tensor(
        out=k2[:, 120:128], in0=k1[:, 120:128], in1=k2[:, 120:128], op=mult
    )
    v.tensor_tensor(
        out=k1[:, 120:128], in0=preds_f[:, 120:128], in1=prev_f[:, 120:128],
        op=ne,
    )
    v.tensor_tensor(
        out=keep_i[:, 120:128], in0=k1[:, 120:128], in1=k2[:, 120:128],
        op=mult,
    )
    nc.scalar.dma_start(out=preds_o[:, 120:128], in_=preds_i[:, 120:128])
    nc.scalar.dma_start(out=mlp_o[:, 120:128], in_=gmax[:, 120:128])
    nc.sync.dma_start(out=keep_o[:, 120:128], in_=keep_i[:, 120:128])


def _host_inputs(log_probs, input_lengths):
    log_probs = np.ascontiguousarray(np.asarray(log_probs, dtype=np.float32))
    lens = np.asarray(input_lengths).astype(np.int64)
    cols = np.arange(NT)
    # t = p*32 + c  (frame index within utterance)
    tvals = (np.arange(P)[:, None] * CPU + (cols % CPU)[None, :]).astype(
        np.float32
    )
    in_maps = []
    for c in range(NCORES):
        lp_c = log_probs[c * BLOC : (c + 1) * BLOC].reshape(F, V)
        ln_c = lens[c * BLOC : (c + 1) * BLOC].astype(np.float32)
        ln_exp = np.broadcast_to(ln_c[cols // CPU][None, :], (P, NT))
        in_maps.append(
            {
                "lp": lp_c,
                "tv": tvals,
                "ln": np.ascontiguousarray(ln_exp, dtype=np.float32),
                "idm": np.eye(P, dtype=np.float32),
            }
        )
    return in_maps


def _grid_to_bt(arr):
    # arr[p, u*32+c] = value for frame t=p*32+c of utterance u
    return arr.reshape(P, BLOC, CPU).transpose(1, 0, 2).reshape(BLOC, T)


def kernel(log_probs, input_lengths, **_kw):
    if "nc" not in _CACHE:
        _CACHE["nc"] = _build_program()
    nc = _CACHE["nc"]
    in_maps = _host_inputs(log_probs, input_lengths)
    res = run_bass_kernel_spmd(nc, in_maps, core_ids=list(range(NCORES)))
    preds = np.empty((B, T), dtype=np.int32)
    keep = np.empty((B, T), dtype=bool)
    max_logp = np.empty((B, T), dtype=np.float32)
    for c, r in enumerate(res.results):
        sl = slice(c * BLOC, (c + 1) * BLOC)
        preds[sl] = _grid_to_bt(r["preds"])
        keep[sl] = _grid_to_bt(r["keep"]).astype(bool)
        max_logp[sl] = _grid_to_bt(r["maxlp"])
    return preds, keep, max_logp
